# revision 43
# baseline (speedup 1.0000x reference)
"""Bass/Trainium2 kernel for nn_EuclideanGraphEncoder (GCN message passing).

Strategy: data-parallel over the batch (4 graphs per core, 8 cores),
weights replicated, no collectives.

Design:
  - fp8 DoubleRow aggregation: the adjacency ships as fp8e4 (x16 scale)
    in a pair-interleaved layout [128, 4, 2, 1024]; each aggregation
    matmul contracts 256 nodes per instruction (MatmulPerfMode.DoubleRow)
    — 4 MMs per 512-col PSUM tile instead of 8. Halves both PE time and
    adjacency DMA bytes. msg tiles are written in fp8e4 directly by the
    DVE bias-add that drains the linear-layer PSUM.
  - The embedding is folded into layer 0 on the host (W0' = We @ Wl0),
    so the device's first linear runs straight off x (K=64).
  - ALL small tensors (W0'+x0|x1, biases pre-broadcast to 128 rows,
    x2|x3, Wl', Wp', fp16 node masks) are packed host-side into ONE
    [128, 4064] fp16 DRAM tensor. EVERY data-bearing DMA rides the SYNC
    HWDGE ring in need order (it spins up ~2us faster and sustains ~3x
    the ACT ring's rate); the ACT ring carries only the bias row and the
    early stores. Few DMAs => the 8 HWDGE completion lanes (a trigger
    blocks on the 8-earlier DMA's completion) wrap benignly, and
    on-device broadcast DMAs (slow DRE replication) are gone entirely.
  - Hand-scheduled head, diagonal-wavefront tail: DMA completion sems
    lag data by 1.5-3us, so the head is paced by the need-ordered sem
    arrival sequence; graph-0's depth chain re-reads its SBUF-resident
    adjacency between later graphs' arrivals, graph-1's aggregation
    emits as halves around it, and graphs 2/3's layer-0 linears slot in
    when their x-data lands. From mid-stream the schedule is PE-bound
    diagonal with 2-event linear lead.
  - HAM clock management: full-K=128 warm-up matmuls (K=1 rank-1s do
    NOT count as PE activity!) hold the 2.4GHz un-throttle from ~10.5us;
    N_HOLD tail matmuls keep K=8/8 through the walrus sem-clear epilogue
    (the PE's ~55-instruction clear slice is the teardown straggler).
  - Projection: 8 chunk matmuls + one rank-1 bias matmul accumulate in
    one PSUM bank; a DVE multiply (broadcast fp16 mask AP) drains it to
    fp16, emitted at slot +1 so the tail drains never queue behind
    layer-2 relu work; tail graphs split drain+store in half across both
    HWDGE rings so the final completion receipts parallelize (out DRAM
    is partition-major; host un-permutes).

Scales (exact powers of two, folded into host-side weights):
  adj8 = fp8(16*adj); msg_dev = msg_true/Sm[i]; h_dev = h_true/Sh[i];
  out = fp16((h3@Wp + bp)/So) * mask;  host returns out*So as f32.
"""

import sys
from contextlib import ExitStack

import numpy as np
import ml_dtypes

try:
    import concourse.bass as bass
except ImportError:  # fall back to the repo checkout
    sys.path.insert(0, "/opt/trn_rl_repo")
    import concourse.bass as bass

import concourse.tile as tile
from concourse import bacc, mybir
from concourse.bass_utils import run_bass_kernel_spmd

B, N, IN_DIM, HID, OUT = 32, 1024, 64, 128, 64
NUM_LAYERS = 3
N_CORES = 8
BPC = B // N_CORES  # graphs per core
NC8 = N // 128      # node chunks of 128
NPAIR = NC8 // 2    # DoubleRow chunk pairs (256 nodes each)

FP8 = mybir.dt.float8e4
FP16 = mybir.dt.float16
FP32 = mybir.dt.float32
RELU = mybir.ActivationFunctionType.Relu
COPY = mybir.ActivationFunctionType.Copy
DR = mybir.MatmulPerfMode.DoubleRow

# numeric scales (see module docstring); all exact powers of two
ADJ_SCALE = 16.0
SM = [2.0 ** -5, 2.0 ** -1, 2.0 ** 7]        # msg_dev = msg_true / SM[i]
SH = [None, 2.0 ** -7, 2.0, 2.0 ** 8]        # h_dev = h_true / SH[i]
SO = 2.0 ** 7                                 # out_dev = out_true / SO
RELU_SCALE = [SM[i] / (ADJ_SCALE * SH[i + 1]) for i in range(3)]

N_WARM = 10       # prologue HAM warm-up matmuls (512 cols each, cold clock)
N_BRIDGE = 2      # PE-activity bridge between prologue linears and agg(0,0)
N_HOLD = 33       # tail matmuls holding K=8/8 through the sem-clear epilogue

# pack column layout (fp16, 128 partitions), ordered so each DMA slice is
# contiguous and need-ordered: A=[W0|x01] on the sync ring ahead of the
# adjacency; B1=[bl0], B2=[x23], B3=[Wl1|bl1], B4=[Wl2|bl2|Wp|masks] on ACT.
PK_W0 = 0          # W0' on partition halves 0:64 AND 64:128, cols 0:128
PK_X01 = 128       # x0 | x1 (parts 0:64 | 64:128), cols 128:1152
PK_BL0 = 1152      # bl0 broadcast row, cols 1152:1664
PK_X23 = 1664      # x2 | x3, cols 1664:2688
PK_WL1 = 2688      # Wl1', cols 2688:2816
PK_BL1 = 2816      # bl1 broadcast row, cols 2816:3328
PK_WL2 = 3328      # Wl2', cols 3328:3456
PK_BL2 = 3456      # bl2 broadcast row, cols 3456:3968
PK_WP = 3968       # Wp', cols 3968:4032
PK_MASK = 4032     # fp16 masks, [128, 8] per graph, cols 4032:4064
PK_C = 4064


def _kernel_body(ctx, tc, out, adj8, pack, rows):
    nc = tc.nc

    consts = ctx.enter_context(tc.tile_pool(name="consts", bufs=1))
    adj_pool = ctx.enter_context(tc.tile_pool(name="adj", bufs=BPC))
    h_pool = ctx.enter_context(tc.tile_pool(name="h", bufs=8))
    msg_pool = ctx.enter_context(tc.tile_pool(name="msg", bufs=6))
    o_pool = ctx.enter_context(tc.tile_pool(name="o", bufs=BPC))
    psA = ctx.enter_context(tc.tile_pool(name="psA", bufs=2, space="PSUM"))
    psM = ctx.enter_context(tc.tile_pool(name="psM", bufs=2, space="PSUM"))
    psO = ctx.enter_context(tc.tile_pool(name="psO", bufs=2, space="PSUM"))

    ones_t = consts.tile([1, HID], FP16, tag="ones")
    warm_t = consts.tile([128, 512], FP16, tag="warm")
    # full-width warm operand: K=1 rank-1 matmuls do NOT register as PE
    # activity for the HAM clock gate — warm-up must drive all 128 rows.
    # memset warm FIRST: the first warm-up LDWEIGHTS waits on it, and
    # every 100ns earlier here moves the 2.4GHz un-throttle point earlier.
    nc.vector.memset(warm_t[:], 0.0)
    nc.vector.memset(ones_t[:], 1.0)

    pack_t = consts.tile([128, PK_C], FP16, tag="pack")
    rows_t = consts.tile([1, 512], FP16, tag="rows")
    # W0' is duplicated on both partition halves (host-side): lhsT and rhs
    # must share a base partition, and odd graphs' x sits on parts 64:128.
    w0_ap2 = [pack_t[0:IN_DIM, PK_W0:PK_W0 + HID],
              pack_t[64:64 + IN_DIM, PK_W0:PK_W0 + HID]]
    wl_ap = [None, pack_t[:, PK_WL1:PK_WL1 + HID], pack_t[:, PK_WL2:PK_WL2 + HID]]
    wp_ap = pack_t[:, PK_WP:PK_WP + OUT]
    bp_row = rows_t[:, 0:512]
    xts = [pack_t[0:64, PK_X01:PK_X01 + N], pack_t[64:128, PK_X01:PK_X01 + N],
           pack_t[0:64, PK_X23:PK_X23 + N], pack_t[64:128, PK_X23:PK_X23 + N]]
    bl_ap = [pack_t[:, PK_BL0:PK_BL0 + 512], pack_t[:, PK_BL1:PK_BL1 + 512],
             pack_t[:, PK_BL2:PK_BL2 + 512]]
    masks = [pack_t[:, PK_MASK + 8 * bb:PK_MASK + 8 * (bb + 1)]
             for bb in range(BPC)]
    adjs = [adj_pool.tile([128, NPAIR, 2, N], FP8, tag="adj", name=f"adj{bb}")
            for bb in range(BPC)]

    # Two HWDGE rings stream in parallel. The sync ring spins up ~2us
    # faster and runs ~2x the ACT ring's rate while both stream — so the
    # layer-0-critical pack head [W0|x01] leads the SYNC ring, ahead of
    # the adjacency; the remaining pack slices ride the ACT ring in need
    # order. 14 DMAs total: the 8 HWDGE completion lanes (a trigger blocks
    # on the 8-earlier DMA's completion) wrap benignly — every such pair
    # completes long before the blocked trigger's ring could start it.
    # No SWDGE at all.
    def adj_q(bb, c2):
        # per-chunk-pair quarter DMAs: a completion sem fires every ~1.1us
        # (vs 2.2 for halves) and each aggregation c2-slice waits only its
        # own quarter — the sem-paced head and mid-phase stalls shrink.
        nc.sync.dma_start(adjs[bb][:, c2:c2 + 1, :, :],
                          adj8[bb, :, c2:c2 + 1, :, :])

    def adj_half(bb, h):
        adj_q(bb, 2 * h)
        adj_q(bb, 2 * h + 1)

    # EVERYTHING data-bearing rides the SYNC ring, in need order: the sync
    # HWDGE ring sustains ~3-4x the ACT ring's rate when both stream, and
    # a single FIFO ring gives exact control of arrival order. The ACT
    # ring carries only the tiny bias row and three mid-kernel stores.
    # 8-apart lane pairs (a trigger waits the 8-earlier DMA's completion)
    # are all early-completers: benign.
    nc.sync.dma_start(pack_t[:, 0:PK_BL0], pack[:, 0:PK_BL0])       # W0|x01
    nc.sync.dma_start(pack_t[:, PK_BL0:PK_X23], pack[:, PK_BL0:PK_X23])
    adj_half(0, 0)
    adj_half(0, 1)
    nc.sync.dma_start(pack_t[:, PK_X23:PK_WL1], pack[:, PK_X23:PK_WL1])
    nc.sync.dma_start(pack_t[:, PK_WL1:PK_WL2], pack[:, PK_WL1:PK_WL2])
    adj_half(1, 0)
    adj_half(1, 1)
    nc.sync.dma_start(pack_t[:, PK_WL2:PK_C], pack[:, PK_WL2:PK_C])
    adj_half(2, 0)
    adj_half(2, 1)
    adj_half(3, 0)
    adj_half(3, 1)
    nc.scalar.dma_start(rows_t[:], rows[:, :])

    # HAM warm-up: dependency-free full-array (K=128) matmuls keep the PE
    # activity window open from ~7us so the 2.4GHz un-throttle fires
    # (~3.4us of sustained busy) before real work.
    warm_ps = psO.tile([128, 512], FP32, tag="psO", name="warm")
    for _ in range(N_WARM):
        nc.tensor.matmul(warm_ps[:], warm_t[:, 0:128], warm_t[:],
                         start=True, stop=True)

    def emit_bridge(n):
        # keep the PE activity window open across waits for adjacency
        # completion sems — a >2us idle re-throttles the HAM clock. Writes
        # the warm PSUM tile (psO slot 0) so psA stays free for the open
        # aggregation accumulation groups.
        for _ in range(n):
            nc.tensor.matmul(warm_ps[:], warm_t[:, 0:128], warm_t[:],
                             start=True, stop=True)

    def emit_linear_mms(bb, i, h):
        # msg[n, k] = (h @ Wl'[i] + bl'[i]) -> fp8 pair layout
        msg_t = msg_pool.tile([128, NPAIR, 2, HID], FP8, tag="msg",
                              name=f"msg{bb}_{i}")
        pms = []
        for half in range(2):
            pm = psM.tile([128, 512], FP32, tag="psM")
            for k in range(4):
                c = 4 * half + k
                if i == 0:
                    lhsT = xts[bb][:, c * 128:(c + 1) * 128]
                    rhs = w0_ap2[bb % 2]
                else:
                    lhsT = h[:, c * 128:(c + 1) * 128]
                    rhs = wl_ap[i]
                nc.tensor.matmul(pm[:, k * 128:(k + 1) * 128], lhsT, rhs,
                                 start=True, stop=True)
            pms.append(pm)
        return msg_t, pms

    def emit_linear_drain(msg_t, pms, i):
        for half in range(2):
            half_ap = msg_t[:, 2 * half:2 * half + 2, :, :]
            nc.vector.tensor_add(
                half_ap.rearrange("p a b c -> p (a b c)"), pms[half][:],
                bl_ap[i][:])

    def emit_linear(bb, i, h):
        msg_t, pms = emit_linear_mms(bb, i, h)
        emit_linear_drain(msg_t, pms, i)
        return msg_t

    def emit_agg_half(bb, msg_t, pa, h):
        # one adjacency half (chunk pairs 2h, 2h+1): the accumulation
        # groups (both t-banks) open at h=0 and close at h=1, so other
        # matmuls (bridge, linears into other banks) can be emitted
        # between the halves while the second half's DMA completes.
        for c2 in (2 * h, 2 * h + 1):
            for t in range(2):
                nc.tensor.matmul(
                    pa[:, t * 512:(t + 1) * 512],
                    msg_t[:, c2, :, :],
                    adjs[bb][:, c2, :, t * 512:(t + 1) * 512],
                    start=(c2 == 0), stop=(c2 == NPAIR - 1), perf_mode=DR,
                    skip_group_check=True)

    def emit_agg(bb, i, msg_t):
        pa = psA.tile([128, N], FP32, tag="psA")
        # c2-major: both t-tiles' matmuls share one weight load per chunk
        # pair; the two banks' accumulation groups interleave.
        emit_agg_half(bb, msg_t, pa, 0)
        emit_agg_half(bb, msg_t, pa, 1)
        return pa

    def emit_relu(bb, i, pa):
        # Layers 0/1: one full-width ACT on scalar — keeps the relu off
        # the DVE, whose queue (msg bias-adds) is the critical chain into
        # the next aggregation. Layer 2: split scalar/DVE — the DVE has
        # no adds left, and the projection matmuls wait on this relu with
        # only a short wavefront cover.
        h2 = h_pool.tile([HID, N], FP16, tag="h", name=f"h{bb}_{i}")
        if i < NUM_LAYERS - 1:
            nc.scalar.activation(h2[:], pa[:], RELU, scale=RELU_SCALE[i])
        else:
            nc.scalar.activation(h2[:, 0:512], pa[:, 0:512], RELU,
                                 scale=RELU_SCALE[i])
            nc.vector.tensor_scalar(h2[:, 512:1024], pa[:, 512:1024],
                                    RELU_SCALE[i], 0.0,
                                    op0=mybir.AluOpType.mult,
                                    op1=mybir.AluOpType.max)
        return h2

    def emit_proj_mms(bb, h):
        po = psO.tile([128, 512], FP32, tag="psO", name=f"psO{bb}")
        for c in range(NC8):
            nc.tensor.matmul(po[:, c * OUT:(c + 1) * OUT],
                             h[:, c * 128:(c + 1) * 128], wp_ap,
                             start=(c == 0), stop=False, skip_group_check=True)
        nc.tensor.matmul(po[:], ones_t[:], bp_row, start=False, stop=True,
                         skip_group_check=True)
        return po

    def emit_proj_drain(bb, po):
        # DVE multiply drains the projection PSUM with the node mask
        # applied via a broadcast AP; contiguous per-partition stores (out
        # DRAM is partition-major [128, NC8, OUT]; the host un-permutes).
        # Tail graphs (2, 3) split drain+store in half across both HWDGE
        # rings: each half's DVE-completion-sem -> trigger -> data ->
        # receipt chain starts ~0.7us earlier and the two receipts
        # parallelize — the teardown barrier waits on the LAST of these.
        o_big = o_pool.tile([128, NC8, OUT], FP16, tag="o", name=f"o{bb}")
        mask_b = masks[bb].unsqueeze(-1).broadcast_to([128, NC8, OUT])
        po3 = po[:].rearrange("p (a b) -> p a b", a=NC8)
        if bb < BPC - 2:
            nc.vector.tensor_tensor(o_big[:], po3, mask_b,
                                    op=mybir.AluOpType.mult)
            nc.scalar.dma_start(out[bb], o_big[:])
            return
        h4 = NC8 // 2
        for h in range(2):
            sl = slice(h * h4, (h + 1) * h4)
            nc.vector.tensor_tensor(o_big[:, sl, :], po3[:, sl, :],
                                    mask_b[:, sl, :], op=mybir.AluOpType.mult)
            ring = nc.sync if (bb + h) % 2 else nc.scalar
            ring.dma_start(out[bb][:, sl, :], o_big[:, sl, :])

    # ---- explicit head schedule ----
    # The head is paced by the sync ring's need-ordered arrival sequence
    # (packA ~10, bl0 ~10.4, a0h0 ~11.8, a0h1 ~13.3, x23 ~14.1, Wl1|bl1
    # ~14.6, a1h0 ~16.1, a1h1 ~17.6, ... sems, ~1.5us apart). Graph 0's
    # depth chain re-reads its SBUF-resident adjacency and fills the PE
    # between later graphs' arrivals; graphs 2/3's layer-0 linears slot in
    # right when their x-data lands. Graph-1's aggregation is emitted as
    # halves with graph-0's layer-1 aggregation between them.
    msgs = [None] * BPC
    hs = [None] * BPC
    pos_tiles = [None] * BPC

    msgs[0] = emit_linear(0, 0, None)
    msgs[1] = emit_linear(1, 0, None)
    emit_bridge(N_BRIDGE)

    pa00 = psA.tile([128, N], FP32, tag="psA", name="pa00")
    emit_agg_half(0, msgs[0], pa00, 0)       # a0h0
    emit_bridge(1)
    emit_agg_half(0, msgs[0], pa00, 1)       # a0h1
    hs[0] = emit_relu(0, 0, pa00)
    msgs[2] = emit_linear(2, 0, None)        # x23 sem lands here
    m01 = emit_linear(0, 1, hs[0])
    pa10 = psA.tile([128, N], FP32, tag="psA", name="pa10")
    emit_agg_half(1, msgs[1], pa10, 0)       # a1h0
    pa = emit_agg(0, 1, m01)                 # adj g0 resident
    hs[0] = emit_relu(0, 1, pa)
    emit_agg_half(1, msgs[1], pa10, 1)       # a1h1
    hs[1] = emit_relu(1, 0, pa10)
    msgs[3] = emit_linear(3, 0, None)
    m02 = emit_linear(0, 2, hs[0])
    m11 = emit_linear(1, 1, hs[1])
    pa = emit_agg(0, 2, m02)
    hs[0] = emit_relu(0, 2, pa)
    pa = emit_agg(1, 1, m11)
    hs[1] = emit_relu(1, 1, pa)
    pa = emit_agg(2, 0, msgs[2])             # a2 halves
    hs[2] = emit_relu(2, 0, pa)

    # ---- diagonal tail ----
    # From here the schedule is purely PE-bound; the diagonal wavefront
    # with 2-event linear lead covers every relu + bias-add latency.
    head = [(0, 0), (0, 1), (1, 0), (0, 2), (1, 1), (2, 0)]
    tail = [(3, 0), (2, 1), (1, 2), (0, 3), (3, 1), (2, 2), (1, 3),
            (3, 2), (2, 3), (3, 3)]
    pos = {ev: k for k, ev in enumerate(head + tail)}
    n_head = len(head)
    lin_slot = {}
    for (g, i) in tail:
        if i in (0, NUM_LAYERS):
            continue
        lin_slot.setdefault(
            max(pos[(g, i - 1)] + 1, pos[(g, i)] - 3, n_head), []).append((g, i))
    # drains at +1: the DVE runs its queue in emission order, so each
    # projection's drain must be emitted as early as its PSUM can be ready
    # — a later slot serializes the tail drains behind layer-2 relu work
    # AND makes the next projection's PSUM-bank WAR wait on this drain.
    drain_slot = {}
    for g in range(BPC):
        drain_slot.setdefault(
            min(pos[(g, NUM_LAYERS)] + 1, len(head + tail)), []).append(g)

    for k, (g, i) in enumerate(tail, start=n_head):
        for (lg, li) in lin_slot.get(k, ()):
            msgs[lg] = emit_linear(lg, li, hs[lg])
        if i < NUM_LAYERS:
            pa = emit_agg(g, i, msgs[g])
            hs[g] = emit_relu(g, i, pa)
        else:
            pos_tiles[g] = emit_proj_mms(g, hs[g])
        for dg in drain_slot.get(k + 1, ()):
            emit_proj_drain(dg, pos_tiles[dg])
    for dg in drain_slot.get(len(head + tail), ()):
        emit_proj_drain(dg, pos_tiles[dg])

    # Hold the HAM K=8/8 clock through the walrus sem-clear epilogue: the
    # PE's ~55-instruction clear slice is the teardown straggler and issues
    # ~2x faster un-throttled. The PE is idle from the last projection to
    # the teardown barrier anyway (the barrier waits on the tail stores'
    # completion receipts) — fill that window with dependency-free matmuls.
    hold_ps = psM.tile([128, 512], FP32, tag="psM", name="hold")
    for _ in range(N_HOLD):
        nc.tensor.matmul(hold_ps[:], warm_t[:, 0:128], warm_t[:],
                         start=True, stop=True)


def build_nc():
    # Bacc (not raw Bass): its compile() runs generate_event_semaphores,
    # which splits multi-sem waits down to the 1-wait-per-instruction
    # hardware limit walrus enforces.
    nc = bacc.Bacc("TRN2", debug=False, num_devices=N_CORES, num_swdge_queues=2)
    adj8 = nc.dram_tensor("adj8", [BPC, 128, NPAIR, 2, N], FP8,
                          kind="ExternalInput").ap()
    pack = nc.dram_tensor("pack", [128, PK_C], FP16, kind="ExternalInput").ap()
    rows = nc.dram_tensor("rows", [1, 512], FP16, kind="ExternalInput").ap()
    out = nc.dram_tensor("out", [BPC, 128, NC8, OUT], FP16,
                         kind="ExternalOutput").ap()

    with tile.TileContext(nc) as tc, ExitStack() as ctx:
        _kernel_body(ctx, tc, out, adj8, pack, rows)
    nc.compile()
    return nc


def make_in_maps(node_features, adjacency_matrix, node_mask, W_embed, Wl, bl,
                 W_proj, b_proj):
    e4 = ml_dtypes.float8_e4m3
    x = np.asarray(node_features, dtype=np.float32)
    adj = np.asarray(adjacency_matrix, dtype=np.float32)
    mask = np.asarray(node_mask, dtype=np.float32)
    We = np.asarray(W_embed, np.float64)
    Wl64 = np.asarray(Wl, np.float64)
    bl64 = np.asarray(bl, np.float64)
    Wp = np.asarray(W_proj, np.float64)
    bp = np.asarray(b_proj, np.float64)

    base = np.zeros((128, PK_C), np.float16)
    w0p = (We @ Wl64[0] / SM[0]).astype(np.float16)
    base[:IN_DIM, PK_W0:PK_W0 + HID] = w0p
    base[64:64 + IN_DIM, PK_W0:PK_W0 + HID] = w0p
    base[:, PK_WL1:PK_WL1 + HID] = (Wl64[1] * (SH[1] / SM[1])).astype(np.float16)
    base[:, PK_WL2:PK_WL2 + HID] = (Wl64[2] * (SH[2] / SM[2])).astype(np.float16)
    base[:, PK_WP:PK_WP + OUT] = (Wp * (SH[3] / SO)).astype(np.float16)
    for i in range(NUM_LAYERS):
        row = np.tile(bl64[i] / SM[i], 4).astype(np.float16)
        off = (PK_BL0, PK_BL1, PK_BL2)[i]
        base[:, off:off + 512] = row[None, :]

    rows_np = np.tile(bp / SO, NC8).astype(np.float16).reshape(1, 512)

    in_maps = []
    for cc in range(N_CORES):
        sl = slice(cc * BPC, (cc + 1) * BPC)
        # adj8[bb, j, c2, o, n] = fp8(16 * adj[n, c2*256 + o*128 + j])
        a = np.ascontiguousarray(adj[sl].transpose(0, 2, 1))  # [BPC, m, n]
        a = a.reshape(BPC, NPAIR, 2, 128, N).transpose(0, 3, 1, 2, 4)
        pk = base.copy()
        pk[:IN_DIM, PK_X01:PK_X01 + N] = x[cc * BPC + 0].T.astype(np.float16)
        pk[64:128, PK_X01:PK_X01 + N] = x[cc * BPC + 1].T.astype(np.float16)
        pk[:IN_DIM, PK_X23:PK_X23 + N] = x[cc * BPC + 2].T.astype(np.float16)
        pk[64:128, PK_X23:PK_X23 + N] = x[cc * BPC + 3].T.astype(np.float16)
        m = mask[sl].reshape(BPC, NC8, 128).transpose(0, 2, 1)  # [BPC,128,NC8]
        for bb in range(BPC):
            pk[:, PK_MASK + 8 * bb:PK_MASK + 8 * (bb + 1)] = m[bb].astype(
                np.float16)
        in_maps.append({
            "adj8": (np.float32(ADJ_SCALE) * a).astype(e4),
            "pack": pk,
            "rows": rows_np,
        })
    return in_maps


_NC_CACHE = None


def get_nc():
    global _NC_CACHE
    if _NC_CACHE is None:
        _NC_CACHE = build_nc()
    return _NC_CACHE


def postprocess(raw_out):
    # device layout [BPC, 128(p), NC8(c), OUT] -> natural [BPC, N, OUT]
    # where n = c*128 + p; then undo the output scale.
    o = np.asarray(raw_out, np.float32).transpose(0, 2, 1, 3)
    return o.reshape(BPC, N, OUT) * np.float32(SO)


def kernel(**inputs):
    nc = get_nc()
    in_maps = make_in_maps(**inputs)
    res = run_bass_kernel_spmd(nc, in_maps, list(range(N_CORES)))
    outs = [postprocess(res.results[c]["out"]) for c in range(N_CORES)]
    return np.concatenate(outs, axis=0)


if __name__ == "__main__":
    rng = np.random.default_rng(0)
    ins = {
        "node_features": rng.standard_normal((B, N, IN_DIM), dtype=np.float32),
        "adjacency_matrix": rng.random((B, N, N), dtype=np.float32),
        "node_mask": np.ones((B, N, 1), np.float32),
        "W_embed": rng.standard_normal((IN_DIM, HID), dtype=np.float32) * 0.1,
        "Wl": rng.standard_normal((NUM_LAYERS, HID, HID), dtype=np.float32) * 0.08,
        "bl": rng.standard_normal((NUM_LAYERS, HID), dtype=np.float32) * 0.08,
        "W_proj": rng.standard_normal((HID, 2 * 32), dtype=np.float32) * 0.08,
        "b_proj": rng.standard_normal((2 * 32,), dtype=np.float32) * 0.08,
    }
    out = kernel(**ins)
    print("out", out.shape, out.dtype, float(np.abs(out).mean()))


# revision 44
# speedup vs baseline: 1.0231x; 1.0231x over previous
"""Bass/Trainium2 kernel for nn_EuclideanGraphEncoder (GCN message passing).

Strategy: data-parallel over the batch (4 graphs per core, 8 cores),
weights replicated, no collectives.

Design:
  - fp8 DoubleRow aggregation: the adjacency ships as fp8e4 (x16 scale)
    in a pair-interleaved layout [128, 4, 2, 1024]; each aggregation
    matmul contracts 256 nodes per instruction (MatmulPerfMode.DoubleRow)
    — 4 MMs per 512-col PSUM tile instead of 8. Halves both PE time and
    adjacency DMA bytes. msg tiles are written in fp8e4 directly by the
    DVE bias-add that drains the linear-layer PSUM.
  - The embedding is folded into layer 0 on the host (W0' = We @ Wl0),
    so the device's first linear runs straight off x (K=64).
  - ALL small tensors (W0'+x0|x1, biases pre-broadcast to 128 rows,
    x2|x3, Wl', Wp', fp16 node masks) are packed host-side into ONE
    [128, 4064] fp16 DRAM tensor. EVERY data-bearing DMA rides the SYNC
    HWDGE ring in need order (it spins up ~2us faster and sustains ~3x
    the ACT ring's rate); the ACT ring carries only the bias row and the
    early stores. Few DMAs => the 8 HWDGE completion lanes (a trigger
    blocks on the 8-earlier DMA's completion) wrap benignly, and
    on-device broadcast DMAs (slow DRE replication) are gone entirely.
  - Hand-scheduled head, diagonal-wavefront tail: DMA completion sems
    lag data by 1.5-3us, so the head is paced by the need-ordered sem
    arrival sequence; graph-0's depth chain re-reads its SBUF-resident
    adjacency between later graphs' arrivals, graph-1's aggregation
    emits as halves around it, and graphs 2/3's layer-0 linears slot in
    when their x-data lands. From mid-stream the schedule is PE-bound
    diagonal with 2-event linear lead.
  - HAM clock management: full-K=128 warm-up matmuls (K=1 rank-1s do
    NOT count as PE activity!) hold the 2.4GHz un-throttle from ~10.5us;
    N_HOLD tail matmuls keep K=8/8 through the walrus sem-clear epilogue
    (the PE's ~55-instruction clear slice is the teardown straggler).
  - Projection: 8 chunk matmuls + one rank-1 bias matmul accumulate in
    one PSUM bank; a DVE multiply (broadcast fp16 mask AP) drains it to
    fp16, emitted at slot +1 so the tail drains never queue behind
    layer-2 relu work; tail graphs split drain+store in half across both
    HWDGE rings so the final completion receipts parallelize (out DRAM
    is partition-major; host un-permutes).

Scales (exact powers of two, folded into host-side weights):
  adj8 = fp8(16*adj); msg_dev = msg_true/Sm[i]; h_dev = h_true/Sh[i];
  out = fp16((h3@Wp + bp)/So) * mask;  host returns out*So as f32.
"""

import sys
from contextlib import ExitStack

import numpy as np
import ml_dtypes

try:
    import concourse.bass as bass
except ImportError:  # fall back to the repo checkout
    sys.path.insert(0, "/opt/trn_rl_repo")
    import concourse.bass as bass

import concourse.tile as tile
from concourse import bacc, mybir
from concourse.bass_utils import run_bass_kernel_spmd

B, N, IN_DIM, HID, OUT = 32, 1024, 64, 128, 64
NUM_LAYERS = 3
N_CORES = 8
BPC = B // N_CORES  # graphs per core
NC8 = N // 128      # node chunks of 128
NPAIR = NC8 // 2    # DoubleRow chunk pairs (256 nodes each)

FP8 = mybir.dt.float8e4
FP16 = mybir.dt.float16
FP32 = mybir.dt.float32
RELU = mybir.ActivationFunctionType.Relu
COPY = mybir.ActivationFunctionType.Copy
DR = mybir.MatmulPerfMode.DoubleRow

# numeric scales (see module docstring); all exact powers of two
ADJ_SCALE = 16.0
SM = [2.0 ** -5, 2.0 ** -1, 2.0 ** 7]        # msg_dev = msg_true / SM[i]
SH = [None, 2.0 ** -7, 2.0, 2.0 ** 8]        # h_dev = h_true / SH[i]
SO = 2.0 ** 7                                 # out_dev = out_true / SO
RELU_SCALE = [SM[i] / (ADJ_SCALE * SH[i + 1]) for i in range(3)]

N_WARM = 10       # prologue HAM warm-up matmuls (512 cols each, cold clock)
N_BRIDGE = 2      # PE-activity bridge between prologue linears and agg(0,0)
N_HOLD = 26       # tail matmuls holding K=8/8 through the sem-clear epilogue

# pack column layout (fp16, 128 partitions), ordered so each DMA slice is
# contiguous and need-ordered: A=[W0|x01] on the sync ring ahead of the
# adjacency; B1=[bl0], B2=[x23], B3=[Wl1|bl1], B4=[Wl2|bl2|Wp|masks] on ACT.
PK_W0 = 0          # W0' on partition halves 0:64 AND 64:128, cols 0:128
PK_X01 = 128       # x0 | x1 (parts 0:64 | 64:128), cols 128:1152
PK_BL0 = 1152      # bl0 broadcast row, cols 1152:1664
PK_X23 = 1664      # x2 | x3, cols 1664:2688
PK_WL1 = 2688      # Wl1', cols 2688:2816
PK_BL1 = 2816      # bl1 broadcast row, cols 2816:3328
PK_WL2 = 3328      # Wl2', cols 3328:3456
PK_BL2 = 3456      # bl2 broadcast row, cols 3456:3968
PK_WP = 3968       # Wp', cols 3968:4032
PK_MASK = 4032     # fp16 masks, [128, 8] per graph, cols 4032:4064
PK_C = 4064


def _kernel_body(ctx, tc, out, adj8, pack, rows):
    nc = tc.nc

    consts = ctx.enter_context(tc.tile_pool(name="consts", bufs=1))
    adj_pool = ctx.enter_context(tc.tile_pool(name="adj", bufs=BPC))
    h_pool = ctx.enter_context(tc.tile_pool(name="h", bufs=8))
    msg_pool = ctx.enter_context(tc.tile_pool(name="msg", bufs=6))
    o_pool = ctx.enter_context(tc.tile_pool(name="o", bufs=BPC))
    psA = ctx.enter_context(tc.tile_pool(name="psA", bufs=2, space="PSUM"))
    psM = ctx.enter_context(tc.tile_pool(name="psM", bufs=2, space="PSUM"))
    psO = ctx.enter_context(tc.tile_pool(name="psO", bufs=2, space="PSUM"))

    ones_t = consts.tile([1, HID], FP16, tag="ones")
    warm_t = consts.tile([128, 512], FP16, tag="warm")
    # full-width warm operand: K=1 rank-1 matmuls do NOT register as PE
    # activity for the HAM clock gate — warm-up must drive all 128 rows.
    # memset warm FIRST: the first warm-up LDWEIGHTS waits on it, and
    # every 100ns earlier here moves the 2.4GHz un-throttle point earlier.
    nc.vector.memset(warm_t[:], 0.0)
    nc.vector.memset(ones_t[:], 1.0)

    pack_t = consts.tile([128, PK_C], FP16, tag="pack")
    rows_t = consts.tile([1, 512], FP16, tag="rows")
    # W0' is duplicated on both partition halves (host-side): lhsT and rhs
    # must share a base partition, and odd graphs' x sits on parts 64:128.
    w0_ap2 = [pack_t[0:IN_DIM, PK_W0:PK_W0 + HID],
              pack_t[64:64 + IN_DIM, PK_W0:PK_W0 + HID]]
    wl_ap = [None, pack_t[:, PK_WL1:PK_WL1 + HID], pack_t[:, PK_WL2:PK_WL2 + HID]]
    wp_ap = pack_t[:, PK_WP:PK_WP + OUT]
    bp_row = rows_t[:, 0:512]
    xts = [pack_t[0:64, PK_X01:PK_X01 + N], pack_t[64:128, PK_X01:PK_X01 + N],
           pack_t[0:64, PK_X23:PK_X23 + N], pack_t[64:128, PK_X23:PK_X23 + N]]
    bl_ap = [pack_t[:, PK_BL0:PK_BL0 + 512], pack_t[:, PK_BL1:PK_BL1 + 512],
             pack_t[:, PK_BL2:PK_BL2 + 512]]
    masks = [pack_t[:, PK_MASK + 8 * bb:PK_MASK + 8 * (bb + 1)]
             for bb in range(BPC)]
    adjs = [adj_pool.tile([128, NPAIR, 2, N], FP8, tag="adj", name=f"adj{bb}")
            for bb in range(BPC)]

    # Two HWDGE rings stream in parallel. The sync ring spins up ~2us
    # faster and runs ~2x the ACT ring's rate while both stream — so the
    # layer-0-critical pack head [W0|x01] leads the SYNC ring, ahead of
    # the adjacency; the remaining pack slices ride the ACT ring in need
    # order. 14 DMAs total: the 8 HWDGE completion lanes (a trigger blocks
    # on the 8-earlier DMA's completion) wrap benignly — every such pair
    # completes long before the blocked trigger's ring could start it.
    # No SWDGE at all.
    def adj_q(bb, c2):
        # per-chunk-pair quarter DMAs: a completion sem fires every ~1.1us
        # (vs 2.2 for halves) and each aggregation c2-slice waits only its
        # own quarter — the sem-paced head and mid-phase stalls shrink.
        nc.sync.dma_start(adjs[bb][:, c2:c2 + 1, :, :],
                          adj8[bb, :, c2:c2 + 1, :, :])

    def adj_half(bb, h):
        adj_q(bb, 2 * h)
        adj_q(bb, 2 * h + 1)

    # EVERYTHING data-bearing rides the SYNC ring, in need order: the sync
    # HWDGE ring sustains ~3-4x the ACT ring's rate when both stream, and
    # a single FIFO ring gives exact control of arrival order. The ACT
    # ring carries only the tiny bias row and three mid-kernel stores.
    # 8-apart lane pairs (a trigger waits the 8-earlier DMA's completion)
    # are all early-completers: benign.
    nc.sync.dma_start(pack_t[:, 0:PK_BL0], pack[:, 0:PK_BL0])       # W0|x01
    nc.sync.dma_start(pack_t[:, PK_BL0:PK_X23], pack[:, PK_BL0:PK_X23])
    adj_half(0, 0)
    adj_half(0, 1)
    nc.sync.dma_start(pack_t[:, PK_X23:PK_WL1], pack[:, PK_X23:PK_WL1])
    nc.sync.dma_start(pack_t[:, PK_WL1:PK_WL2], pack[:, PK_WL1:PK_WL2])
    adj_half(1, 0)
    adj_half(1, 1)
    nc.sync.dma_start(pack_t[:, PK_WL2:PK_C], pack[:, PK_WL2:PK_C])
    adj_half(2, 0)
    adj_half(2, 1)
    adj_half(3, 0)
    adj_half(3, 1)
    nc.scalar.dma_start(rows_t[:], rows[:, :])

    # HAM warm-up: dependency-free full-array (K=128) matmuls keep the PE
    # activity window open from ~7us so the 2.4GHz un-throttle fires
    # (~3.4us of sustained busy) before real work.
    warm_ps = psO.tile([128, 512], FP32, tag="psO", name="warm")
    for _ in range(N_WARM):
        nc.tensor.matmul(warm_ps[:], warm_t[:, 0:128], warm_t[:],
                         start=True, stop=True)

    def emit_bridge(n):
        # keep the PE activity window open across waits for adjacency
        # completion sems — a >2us idle re-throttles the HAM clock. Writes
        # the warm PSUM tile (psO slot 0) so psA stays free for the open
        # aggregation accumulation groups.
        for _ in range(n):
            nc.tensor.matmul(warm_ps[:], warm_t[:, 0:128], warm_t[:],
                             start=True, stop=True)

    def emit_linear_mms(bb, i, h):
        # msg[n, k] = (h @ Wl'[i] + bl'[i]) -> fp8 pair layout
        msg_t = msg_pool.tile([128, NPAIR, 2, HID], FP8, tag="msg",
                              name=f"msg{bb}_{i}")
        pms = []
        for half in range(2):
            pm = psM.tile([128, 512], FP32, tag="psM")
            for k in range(4):
                c = 4 * half + k
                if i == 0:
                    lhsT = xts[bb][:, c * 128:(c + 1) * 128]
                    rhs = w0_ap2[bb % 2]
                else:
                    lhsT = h[:, c * 128:(c + 1) * 128]
                    rhs = wl_ap[i]
                nc.tensor.matmul(pm[:, k * 128:(k + 1) * 128], lhsT, rhs,
                                 start=True, stop=True)
            pms.append(pm)
        return msg_t, pms

    def emit_linear_drain(msg_t, pms, i):
        for half in range(2):
            half_ap = msg_t[:, 2 * half:2 * half + 2, :, :]
            nc.vector.tensor_add(
                half_ap.rearrange("p a b c -> p (a b c)"), pms[half][:],
                bl_ap[i][:])

    def emit_linear(bb, i, h):
        msg_t, pms = emit_linear_mms(bb, i, h)
        emit_linear_drain(msg_t, pms, i)
        return msg_t

    def emit_agg_half(bb, msg_t, pa, h):
        # one adjacency half (chunk pairs 2h, 2h+1): the accumulation
        # groups (both t-banks) open at h=0 and close at h=1, so other
        # matmuls (bridge, linears into other banks) can be emitted
        # between the halves while the second half's DMA completes.
        for c2 in (2 * h, 2 * h + 1):
            for t in range(2):
                nc.tensor.matmul(
                    pa[:, t * 512:(t + 1) * 512],
                    msg_t[:, c2, :, :],
                    adjs[bb][:, c2, :, t * 512:(t + 1) * 512],
                    start=(c2 == 0), stop=(c2 == NPAIR - 1), perf_mode=DR,
                    skip_group_check=True)

    def emit_agg(bb, i, msg_t):
        pa = psA.tile([128, N], FP32, tag="psA")
        # c2-major: both t-tiles' matmuls share one weight load per chunk
        # pair; the two banks' accumulation groups interleave.
        emit_agg_half(bb, msg_t, pa, 0)
        emit_agg_half(bb, msg_t, pa, 1)
        return pa

    def emit_relu(bb, i, pa):
        # Layers 0/1: one full-width ACT on scalar — keeps the relu off
        # the DVE, whose queue (msg bias-adds) is the critical chain into
        # the next aggregation. Layer 2: split scalar/DVE — the DVE has
        # no adds left, and the projection matmuls wait on this relu with
        # only a short wavefront cover.
        h2 = h_pool.tile([HID, N], FP16, tag="h", name=f"h{bb}_{i}")
        if i < NUM_LAYERS - 1:
            nc.scalar.activation(h2[:], pa[:], RELU, scale=RELU_SCALE[i])
        else:
            nc.scalar.activation(h2[:, 0:512], pa[:, 0:512], RELU,
                                 scale=RELU_SCALE[i])
            nc.vector.tensor_scalar(h2[:, 512:1024], pa[:, 512:1024],
                                    RELU_SCALE[i], 0.0,
                                    op0=mybir.AluOpType.mult,
                                    op1=mybir.AluOpType.max)
        return h2

    def emit_proj_mms(bb, h):
        po = psO.tile([128, 512], FP32, tag="psO", name=f"psO{bb}")
        for c in range(NC8):
            nc.tensor.matmul(po[:, c * OUT:(c + 1) * OUT],
                             h[:, c * 128:(c + 1) * 128], wp_ap,
                             start=(c == 0), stop=False, skip_group_check=True)
        nc.tensor.matmul(po[:], ones_t[:], bp_row, start=False, stop=True,
                         skip_group_check=True)
        return po

    def emit_proj_drain(bb, po):
        # DVE multiply drains the projection PSUM with the node mask
        # applied via a broadcast AP; contiguous per-partition stores (out
        # DRAM is partition-major [128, NC8, OUT]; the host un-permutes).
        # Tail graphs (2, 3) split drain+store in half across both HWDGE
        # rings: each half's DVE-completion-sem -> trigger -> data ->
        # receipt chain starts ~0.7us earlier and the two receipts
        # parallelize — the teardown barrier waits on the LAST of these.
        o_big = o_pool.tile([128, NC8, OUT], FP16, tag="o", name=f"o{bb}")
        mask_b = masks[bb].unsqueeze(-1).broadcast_to([128, NC8, OUT])
        po3 = po[:].rearrange("p (a b) -> p a b", a=NC8)
        if bb < BPC - 2:
            nc.vector.tensor_tensor(o_big[:], po3, mask_b,
                                    op=mybir.AluOpType.mult)
            nc.scalar.dma_start(out[bb], o_big[:])
            return
        h4 = NC8 // 2
        for h in range(2):
            sl = slice(h * h4, (h + 1) * h4)
            nc.vector.tensor_tensor(o_big[:, sl, :], po3[:, sl, :],
                                    mask_b[:, sl, :], op=mybir.AluOpType.mult)
            ring = nc.sync if (bb + h) % 2 else nc.scalar
            ring.dma_start(out[bb][:, sl, :], o_big[:, sl, :])

    # ---- explicit head schedule ----
    # The head is paced by the sync ring's need-ordered arrival sequence
    # (packA ~10, bl0 ~10.4, a0h0 ~11.8, a0h1 ~13.3, x23 ~14.1, Wl1|bl1
    # ~14.6, a1h0 ~16.1, a1h1 ~17.6, ... sems, ~1.5us apart). Graph 0's
    # depth chain re-reads its SBUF-resident adjacency and fills the PE
    # between later graphs' arrivals; graphs 2/3's layer-0 linears slot in
    # right when their x-data lands. Graph-1's aggregation is emitted as
    # halves with graph-0's layer-1 aggregation between them.
    msgs = [None] * BPC
    hs = [None] * BPC
    pos_tiles = [None] * BPC

    msgs[0] = emit_linear(0, 0, None)
    msgs[1] = emit_linear(1, 0, None)
    emit_bridge(N_BRIDGE)

    pa00 = psA.tile([128, N], FP32, tag="psA", name="pa00")
    emit_agg_half(0, msgs[0], pa00, 0)       # a0h0
    emit_bridge(1)
    emit_agg_half(0, msgs[0], pa00, 1)       # a0h1
    hs[0] = emit_relu(0, 0, pa00)
    msgs[2] = emit_linear(2, 0, None)        # x23 sem lands here
    m01 = emit_linear(0, 1, hs[0])
    pa10 = psA.tile([128, N], FP32, tag="psA", name="pa10")
    emit_agg_half(1, msgs[1], pa10, 0)       # a1h0
    pa = emit_agg(0, 1, m01)                 # adj g0 resident
    hs[0] = emit_relu(0, 1, pa)
    emit_agg_half(1, msgs[1], pa10, 1)       # a1h1
    hs[1] = emit_relu(1, 0, pa10)
    msgs[3] = emit_linear(3, 0, None)
    m02 = emit_linear(0, 2, hs[0])
    m11 = emit_linear(1, 1, hs[1])
    pa = emit_agg(0, 2, m02)
    hs[0] = emit_relu(0, 2, pa)
    pa = emit_agg(1, 1, m11)
    hs[1] = emit_relu(1, 1, pa)
    pa = emit_agg(2, 0, msgs[2])             # a2 halves
    hs[2] = emit_relu(2, 0, pa)

    # ---- diagonal tail ----
    # From here the schedule is purely PE-bound; the diagonal wavefront
    # with 2-event linear lead covers every relu + bias-add latency.
    head = [(0, 0), (0, 1), (1, 0), (0, 2), (1, 1), (2, 0)]
    tail = [(3, 0), (2, 1), (1, 2), (0, 3), (3, 1), (2, 2), (1, 3),
            (3, 2), (2, 3), (3, 3)]
    pos = {ev: k for k, ev in enumerate(head + tail)}
    n_head = len(head)
    lin_slot = {}
    for (g, i) in tail:
        if i in (0, NUM_LAYERS):
            continue
        lin_slot.setdefault(
            max(pos[(g, i - 1)] + 1, pos[(g, i)] - 2, n_head), []).append((g, i))
    # drains at +1: the DVE runs its queue in emission order, so each
    # projection's drain must be emitted as early as its PSUM can be ready
    # — a later slot serializes the tail drains behind layer-2 relu work
    # AND makes the next projection's PSUM-bank WAR wait on this drain.
    drain_slot = {}
    for g in range(BPC):
        drain_slot.setdefault(
            min(pos[(g, NUM_LAYERS)] + 1, len(head + tail)), []).append(g)

    for k, (g, i) in enumerate(tail, start=n_head):
        for (lg, li) in lin_slot.get(k, ()):
            msgs[lg] = emit_linear(lg, li, hs[lg])
        if i < NUM_LAYERS:
            pa = emit_agg(g, i, msgs[g])
            hs[g] = emit_relu(g, i, pa)
        else:
            pos_tiles[g] = emit_proj_mms(g, hs[g])
        for dg in drain_slot.get(k + 1, ()):
            emit_proj_drain(dg, pos_tiles[dg])
    for dg in drain_slot.get(len(head + tail), ()):
        emit_proj_drain(dg, pos_tiles[dg])

    # Hold the HAM K=8/8 clock through the walrus sem-clear epilogue: the
    # PE's ~55-instruction clear slice is the teardown straggler and issues
    # ~2x faster un-throttled. The PE is idle from the last projection to
    # the teardown barrier anyway (the barrier waits on the tail stores'
    # completion receipts) — fill that window with dependency-free matmuls.
    hold_ps = psM.tile([128, 512], FP32, tag="psM", name="hold")
    for _ in range(N_HOLD):
        nc.tensor.matmul(hold_ps[:], warm_t[:, 0:128], warm_t[:],
                         start=True, stop=True)


def build_nc():
    # Bacc (not raw Bass): its compile() runs generate_event_semaphores,
    # which splits multi-sem waits down to the 1-wait-per-instruction
    # hardware limit walrus enforces.
    nc = bacc.Bacc("TRN2", debug=False, num_devices=N_CORES, num_swdge_queues=2)
    adj8 = nc.dram_tensor("adj8", [BPC, 128, NPAIR, 2, N], FP8,
                          kind="ExternalInput").ap()
    pack = nc.dram_tensor("pack", [128, PK_C], FP16, kind="ExternalInput").ap()
    rows = nc.dram_tensor("rows", [1, 512], FP16, kind="ExternalInput").ap()
    out = nc.dram_tensor("out", [BPC, 128, NC8, OUT], FP16,
                         kind="ExternalOutput").ap()

    with tile.TileContext(nc) as tc, ExitStack() as ctx:
        _kernel_body(ctx, tc, out, adj8, pack, rows)
    nc.compile()
    return nc


def make_in_maps(node_features, adjacency_matrix, node_mask, W_embed, Wl, bl,
                 W_proj, b_proj):
    e4 = ml_dtypes.float8_e4m3
    x = np.asarray(node_features, dtype=np.float32)
    adj = np.asarray(adjacency_matrix, dtype=np.float32)
    mask = np.asarray(node_mask, dtype=np.float32)
    We = np.asarray(W_embed, np.float64)
    Wl64 = np.asarray(Wl, np.float64)
    bl64 = np.asarray(bl, np.float64)
    Wp = np.asarray(W_proj, np.float64)
    bp = np.asarray(b_proj, np.float64)

    base = np.zeros((128, PK_C), np.float16)
    w0p = (We @ Wl64[0] / SM[0]).astype(np.float16)
    base[:IN_DIM, PK_W0:PK_W0 + HID] = w0p
    base[64:64 + IN_DIM, PK_W0:PK_W0 + HID] = w0p
    base[:, PK_WL1:PK_WL1 + HID] = (Wl64[1] * (SH[1] / SM[1])).astype(np.float16)
    base[:, PK_WL2:PK_WL2 + HID] = (Wl64[2] * (SH[2] / SM[2])).astype(np.float16)
    base[:, PK_WP:PK_WP + OUT] = (Wp * (SH[3] / SO)).astype(np.float16)
    for i in range(NUM_LAYERS):
        row = np.tile(bl64[i] / SM[i], 4).astype(np.float16)
        off = (PK_BL0, PK_BL1, PK_BL2)[i]
        base[:, off:off + 512] = row[None, :]

    rows_np = np.tile(bp / SO, NC8).astype(np.float16).reshape(1, 512)

    in_maps = []
    for cc in range(N_CORES):
        sl = slice(cc * BPC, (cc + 1) * BPC)
        # adj8[bb, j, c2, o, n] = fp8(16 * adj[n, c2*256 + o*128 + j])
        a = np.ascontiguousarray(adj[sl].transpose(0, 2, 1))  # [BPC, m, n]
        a = a.reshape(BPC, NPAIR, 2, 128, N).transpose(0, 3, 1, 2, 4)
        pk = base.copy()
        pk[:IN_DIM, PK_X01:PK_X01 + N] = x[cc * BPC + 0].T.astype(np.float16)
        pk[64:128, PK_X01:PK_X01 + N] = x[cc * BPC + 1].T.astype(np.float16)
        pk[:IN_DIM, PK_X23:PK_X23 + N] = x[cc * BPC + 2].T.astype(np.float16)
        pk[64:128, PK_X23:PK_X23 + N] = x[cc * BPC + 3].T.astype(np.float16)
        m = mask[sl].reshape(BPC, NC8, 128).transpose(0, 2, 1)  # [BPC,128,NC8]
        for bb in range(BPC):
            pk[:, PK_MASK + 8 * bb:PK_MASK + 8 * (bb + 1)] = m[bb].astype(
                np.float16)
        in_maps.append({
            "adj8": (np.float32(ADJ_SCALE) * a).astype(e4),
            "pack": pk,
            "rows": rows_np,
        })
    return in_maps


_NC_CACHE = None


def get_nc():
    global _NC_CACHE
    if _NC_CACHE is None:
        _NC_CACHE = build_nc()
    return _NC_CACHE


def postprocess(raw_out):
    # device layout [BPC, 128(p), NC8(c), OUT] -> natural [BPC, N, OUT]
    # where n = c*128 + p; then undo the output scale.
    o = np.asarray(raw_out, np.float32).transpose(0, 2, 1, 3)
    return o.reshape(BPC, N, OUT) * np.float32(SO)


def kernel(**inputs):
    nc = get_nc()
    in_maps = make_in_maps(**inputs)
    res = run_bass_kernel_spmd(nc, in_maps, list(range(N_CORES)))
    outs = [postprocess(res.results[c]["out"]) for c in range(N_CORES)]
    return np.concatenate(outs, axis=0)


if __name__ == "__main__":
    rng = np.random.default_rng(0)
    ins = {
        "node_features": rng.standard_normal((B, N, IN_DIM), dtype=np.float32),
        "adjacency_matrix": rng.random((B, N, N), dtype=np.float32),
        "node_mask": np.ones((B, N, 1), np.float32),
        "W_embed": rng.standard_normal((IN_DIM, HID), dtype=np.float32) * 0.1,
        "Wl": rng.standard_normal((NUM_LAYERS, HID, HID), dtype=np.float32) * 0.08,
        "bl": rng.standard_normal((NUM_LAYERS, HID), dtype=np.float32) * 0.08,
        "W_proj": rng.standard_normal((HID, 2 * 32), dtype=np.float32) * 0.08,
        "b_proj": rng.standard_normal((2 * 32,), dtype=np.float32) * 0.08,
    }
    out = kernel(**ins)
    print("out", out.shape, out.dtype, float(np.abs(out).mean()))
